# revision 5
# baseline (speedup 1.0000x reference)
"""Trainium2 Bass kernel for nn_DelayedSelfAttention (B=4, T=1024, C=1024, H=16).

Sharding: 8 cores = 4 batches x 2 head-groups.  Core c handles batch c//2
and heads [8r, 8r+8) (r = c%2).  Each core computes Q/K/V for its 8 heads
over the full 2T sequence (no duplicated projection work), attention for
its heads over all 2T query rows, and a PARTIAL output projection for all
2T rows using its heads' 512 columns of y (plus its share of the proj-LoRA
mid).  A pairwise ReduceScatter(add) then sums the two partials and hands
core 2b the e1 rows and core 2b+1 the e2 rows — the collective's rank
order does the role split, so the SPMD program has no role-dependent
addressing at all (masks are identical on every core).

Attention runs in the S^T orientation (keys on partitions, queries on the
free axis): no transposes anywhere.  exp on ScalarE, multiplicative {0,1}
masks on boundary tiles only, AV via V augmented with a ones column so
the softmax denominator accumulates as row 64 of the [65, q] matmul
output.  K^T/V/Q^T stay SBUF-resident (no DRAM spill).  Everything is
bf16 except the f32 PSUM accumulations and the rank-8 LoRA pipes (f32r).
Softmax skips max-subtraction (scores are O(1) by construction).
"""

import contextlib
import sys

for _p in ("/opt/trn_rl_repo", "/root/.axon_site/_ro/trn_rl_repo"):
    if _p not in sys.path:
        sys.path.insert(0, _p)

import ml_dtypes
import numpy as np

import concourse.bass as bass
import concourse.mybir as mybir
import concourse.tile as tile_mod
from concourse.bass_utils import run_bass_kernel_spmd
from concourse.tile import TileContext
from concourse.vector_clock import ScopedClock

# ---------------------------------------------------------------------------
# Workaround: this walrus build supports a single semaphore wait per
# instruction.  Split multi-wait instructions into same-engine NoOps each
# carrying one wait (identical sequencer semantics).
# ---------------------------------------------------------------------------
_ws_counter = [0]


def _fresh_name():
    _ws_counter[0] += 1
    return f"I-waitsplit-{_ws_counter[0]}"


def _split_inst_waits(inst):
    si = inst.sync_info
    if si is None:
        return []
    waits = list(si.on_wait or [])
    if len(waits) <= 1:
        return []
    nops = []
    for w in waits[:-1]:
        nop = mybir.InstNoOp(name=_fresh_name())
        nop.engine = inst.engine
        nop.sync_info = mybir.SyncInfo(on_wait=[w], on_update=[])
        nops.append(nop)
    inst.sync_info = mybir.SyncInfo(
        on_wait=[waits[-1]], on_update=list(si.on_update or [])
    )
    return nops


_orig_lower = tile_mod.TileContext._lower_ordered_insts


def _patched_lower(self, ordered):
    for bb_name in list(ordered.keys()):
        new = []
        for inst in ordered[bb_name]:
            new.extend(_split_inst_waits(inst))
            new.append(inst)
        ordered[bb_name] = new
    return _orig_lower(self, ordered)


def _patched_drain_and_barrier(self, tick_clock, wait_clock):
    nc = self.nc
    drain_inst = nc.sync.drain()
    wait_clock.add_sem_waits(
        drain_inst.ins, ScopedClock({None: tick_clock.global_clock})
    )
    nops = _split_inst_waits(drain_inst.ins)
    if nops:
        first_wait = drain_inst.ins.sync_info
        drain_inst.ins.sync_info = mybir.SyncInfo(on_wait=[], on_update=[])
        for nop in nops:
            n2 = nc.sync.nop(nofuse=True)
            n2.ins.sync_info = nop.sync_info
        d2 = nc.sync.drain()
        d2.ins.sync_info = first_wait

    nc.all_engine_barrier()
    assert self.sems is not None
    popped = nc._tile_sem_poison_stack.pop()
    assert popped is self._sem_poison
    nc.clear_and_free_semaphores(list(self.sems.allocated().values()))
    nc.all_engine_barrier()


def _apply_tile_patch():
    if tile_mod.TileContext._lower_ordered_insts is not _patched_lower:
        tile_mod.TileContext._lower_ordered_insts = _patched_lower
        tile_mod.TileContext._drain_and_barrier = _patched_drain_and_barrier


# ---------------------------------------------------------------------------
# Problem constants (hardcoded per the task contract).
# ---------------------------------------------------------------------------
B, T, C, H = 4, 1024, 1024, 16
D = C // H  # 64
SEQ = 2 * T
LOOKAHEAD, OVERLAP = 64, 64
RANK, ALPHA = 8, 16.0
LSCALE = ALPHA / RANK  # 2.0
QSCALE = 1.0 / np.sqrt(D)  # 1/8
NCH = C // 128  # 8 contraction chunks
NMH = 4  # my-head 128-chunks (512 cols / 128)
F32 = mybir.dt.float32
F32R = mybir.dt.float32r
BF16 = mybir.dt.bfloat16

REPLICA_GROUPS = [[0, 1], [2, 3], [4, 5], [6, 7]]


# Trace-time tiling structure, shared by host (mask packing) and device.
# half: 0 = e1 queries, 1 = e2 queries.  qt/j are local 128-tiles (0..7).
def _ktiles_for_block(half, qb):
    """k-tiles (region, j) touched by q-subtiles [4qb, 4qb+4) of `half`."""
    qts = range(4 * qb, 4 * qb + 4)
    if half == 0:
        e1 = sorted({j for qt in qts for j in (qt - 1, qt) if 0 <= j < 8})
    else:
        e1 = sorted({j for qt in qts for j in (qt - 1, qt, qt + 1) if 0 <= j < 8})
    e2 = sorted({j for qt in qts for j in range(qt + 1)})
    return [("e1", j) for j in e1] + [("e2", j) for j in e2]


def _active_qts(half, region, j, qb):
    if region == "e1":
        cand = {j, j + 1} if half == 0 else {j - 1, j, j + 1}
    else:
        cand = set(range(j, 8))
    qts = sorted(cand & set(range(4 * qb, 4 * qb + 4)))
    assert qts == list(range(qts[0], qts[-1] + 1))
    return qts


def _mask_tiles():
    out = []
    for half in (0, 1):
        for qt in range(8):
            if half == 0:
                for j in (qt - 1, qt):
                    if 0 <= j < 8:
                        out.append((half, "e1", j, qt))
                for j in (qt - 1, qt):
                    if j >= 0:
                        out.append((half, "e2", j, qt))
            else:
                for j in (qt - 1, qt, qt + 1):
                    if 0 <= j < 8:
                        out.append((half, "e1", j, qt))
                out.append((half, "e2", qt, qt))
    return out


MASK_TILES = _mask_tiles()  # 60 tiles
MASK_IDX = {k: i for i, k in enumerate(MASK_TILES)}
NMASK = len(MASK_TILES)


def _accum(nc, out_ps, pairs):
    """Accumulating matmul group: list of (lhsT, rhs) into one psum tile."""
    n = len(pairs)
    for i, (lh, rh) in enumerate(pairs):
        nc.tensor.matmul(out_ps, lh, rh, start=(i == 0), stop=(i == n - 1))


# ---------------------------------------------------------------------------
# Device program (identical on all 8 cores; role differences live in data
# and in the ReduceScatter rank order).
# ---------------------------------------------------------------------------
def _build_program():
    _apply_tile_patch()
    nc = bass.Bass("TRN2", target_bir_lowering=False, debug=False, num_devices=8)

    def din(name, shape, dt=F32R):
        return nc.dram_tensor(name, list(shape), dt, kind="ExternalInput").ap()

    xT = din("xT", (C, SEQ), dt=BF16)
    wqk = din("wqk", (C, 1024), dt=BF16)  # [q my512 (prescaled 1/8) | k my512]
    wv = din("wv", (C, 512), dt=BF16)
    la_attn = din("la_attn", (C, RANK), dt=BF16)
    lb_qk = din("lb_qk", (RANK, 1024))  # scaled; q part also 1/8
    lb_v = din("lb_v", (RANK, 512))
    la_proj = din("la_proj", (512, RANK), dt=BF16)  # rows = my 512 y-cols
    lb_proj = din("lb_proj", (RANK, C))
    wproj = din("wproj", (512, C), dt=BF16)  # rows = my 512 y-cols
    masks = din("masks", (NMASK, 128, 128), dt=BF16)
    ones1 = din("ones1", (1, 128))
    yout = nc.dram_tensor("yout", [T, C], F32, kind="ExternalOutput").ap()

    with TileContext(nc) as tc:
        ctx = contextlib.ExitStack()
        with ctx:
            ctx.enter_context(
                nc.allow_low_precision(reason="float32r is full-width fp32 storage")
            )
            # DRAM staging for the pairwise partial-output ReduceScatter.
            dpool = ctx.enter_context(tc.tile_pool(name="dram", bufs=1, space="DRAM"))
            d_in = [
                dpool.tile([2, 512, C], BF16, name=f"d_in{k}") for k in range(2)
            ]
            d_out = [
                dpool.tile([512, C], BF16, name=f"d_out{k}") for k in range(2)
            ]

            # --- persistent SBUF ---
            persist = ctx.enter_context(tc.tile_pool(name="persist", bufs=1))
            qT_sb = persist.tile([128, NMH, SEQ], BF16)
            ktT_sb = persist.tile([128, NMH, SEQ], BF16)
            v_sb = persist.tile([128, 16, 8, D + 1], BF16)  # (part, kt, head, d+1)
            y_acc = persist.tile([128, NMH, SEQ], BF16)  # y^T, my heads
            la_attn_sb = persist.tile([128, NCH, RANK], BF16)
            lb_qk_sb = persist.tile([RANK, 1024], F32R)
            lb_v_sb = persist.tile([RANK, 512], F32R)
            la_proj_sb = persist.tile([128, NMH, RANK], BF16)
            lb_proj_sb = persist.tile([RANK, C], F32R)
            ones1_sb = persist.tile([1, 128], F32R)
            mask_sb = persist.tile([128, NMASK, 128], BF16)
            tmp_kv_sb = persist.tile([RANK, T], F32R)  # attn-lora mid, e2 rows

            nc.sync.dma_start(
                out=la_attn_sb[:], in_=la_attn.rearrange("(ch p) r -> p ch r", p=128)
            )
            nc.sync.dma_start(out=lb_qk_sb[:], in_=lb_qk[:])
            nc.sync.dma_start(out=lb_v_sb[:], in_=lb_v[:])
            nc.sync.dma_start(
                out=la_proj_sb[:], in_=la_proj.rearrange("(ch p) r -> p ch r", p=128)
            )
            nc.sync.dma_start(out=lb_proj_sb[:], in_=lb_proj[:])
            nc.sync.dma_start(out=ones1_sb[:], in_=ones1[:])

            # Ones column of the augmented V (denominator accumulator rows).
            nc.gpsimd.memset(v_sb[:, :, :, D : D + 1], 1.0)

            # --- PSUM pools ---
            ps_s = ctx.enter_context(tc.tile_pool(name="ps_s", bufs=3, space="PSUM"))
            ps_y = ctx.enter_context(tc.tile_pool(name="ps_y", bufs=5, space="PSUM"))
            ps_misc = ps_s

            stage = ctx.enter_context(tc.tile_pool(name="stage", bufs=4))
            rpool = ctx.enter_context(tc.tile_pool(name="rpool", bufs=2))
            small = ctx.enter_context(tc.tile_pool(name="small", bufs=3))
            pt_pool = ctx.enter_context(tc.tile_pool(name="pt", bufs=6))

            # ====== Phase A: Q^T / K^T / V for my heads, full 2T, resident ==
            wqk_ctx = tc.tile_pool(name="wqk_pool", bufs=1)
            xa_ctx = tc.tile_pool(name="xa", bufs=2)
            with wqk_ctx as wqk_pool, xa_ctx as xa_pool:
                wqk_sb = wqk_pool.tile([128, NCH, 1024], BF16)
                wv_sb = wqk_pool.tile([128, NCH, 512], BF16)
                for ch in range(NCH):
                    nc.sync.dma_start(
                        out=wqk_sb[:, ch, :],
                        in_=wqk[128 * ch : 128 * (ch + 1), :],
                    )
                for ch in range(NCH):
                    nc.sync.dma_start(
                        out=wv_sb[:, ch, :],
                        in_=wv[128 * ch : 128 * (ch + 1), :],
                    )

                def do_sblock(s):
                    sl = slice(s * 512, (s + 1) * 512)
                    xt_s = xa_pool.tile([128, NCH, 512], BF16, tag="xa")
                    for ch in range(NCH):
                        nc.sync.dma_start(
                            out=xt_s[:, ch, :],
                            in_=xT[128 * ch : 128 * (ch + 1), sl],
                        )
                    tsl = None
                    if s >= 2:  # e2 rows: attn-lora mid  tmp^T = A^T x
                        tsl = slice((s - 2) * 512, (s - 1) * 512)
                        tmp_ps = ps_misc.tile([RANK, 512], F32, tag="s")
                        _accum(
                            nc,
                            tmp_ps[:],
                            [
                                (la_attn_sb[:, ch, :], xt_s[:, ch, :])
                                for ch in range(NCH)
                            ],
                        )
                        nc.vector.tensor_copy(tmp_kv_sb[:, tsl], tmp_ps[:])
                    for m in range(NMH):  # K^T cols (my heads)
                        cols = slice(512 + 128 * m, 512 + 128 * (m + 1))
                        kps = ps_s.tile([128, 512], F32, tag="s")
                        mms = [
                            (wqk_sb[:, ch, cols], xt_s[:, ch, :]) for ch in range(NCH)
                        ]
                        if s >= 2:
                            mms.append((lb_qk_sb[:, cols], tmp_kv_sb[:, tsl]))
                        _accum(nc, kps[:], mms)
                        nc.vector.tensor_copy(ktT_sb[:, m, sl], kps[:])
                    for m in range(NMH):  # Q^T (prescaled by 1/8 via wqk/lb data)
                        cols = slice(128 * m, 128 * (m + 1))
                        qps = ps_s.tile([128, 512], F32, tag="s")
                        mms = [
                            (wqk_sb[:, ch, cols], xt_s[:, ch, :]) for ch in range(NCH)
                        ]
                        if s >= 2:
                            mms.append((lb_qk_sb[:, cols], tmp_kv_sb[:, tsl]))
                        _accum(nc, qps[:], mms)
                        nc.vector.tensor_copy(qT_sb[:, m, sl], qps[:])
                    for st in range(4):  # V rows (128-row seq tiles)
                        vps = ps_s.tile([128, 512], F32, tag="s")
                        mms = [
                            (
                                xt_s[:, ch, 128 * st : 128 * (st + 1)],
                                wv_sb[:, ch, :],
                            )
                            for ch in range(NCH)
                        ]
                        if s >= 2:
                            base = (s - 2) * 512 + 128 * st
                            mms.append((tmp_kv_sb[:, base : base + 128], lb_v_sb[:]))
                        _accum(nc, vps[:], mms)
                        nc.vector.tensor_copy(
                            v_sb[:, 4 * s + st, :, 0:D],
                            vps[:].rearrange("p (h d) -> p h d", h=8),
                        )

                for s in (0, 1, 2):
                    do_sblock(s)
                nc.sync.dma_start(
                    out=mask_sb[:], in_=masks.rearrange("t p q -> p t q")
                )

                # ===== Phase B/C helpers =====
                wproj_sb = persist.tile([128, NMH, C], BF16)
                for m in range(NMH):
                    nc.sync.dma_start(
                        out=wproj_sb[:, m, :],
                        in_=wproj[128 * m : 128 * (m + 1), :],
                    )

                def _emit_division(half, qb, hp, yus):
                    hoff = half * T + qb * 512
                    for hi in range(2):
                        yu = yus[hi]
                        ysb = pt_pool.tile(
                            [D + 1, 512], F32R, tag="ysb",
                            name=f"ysb_{half}_{qb}_{hp}_{hi}",
                        )
                        nc.vector.tensor_copy(ysb[:], yu[:])
                        r_tmp = small.tile([1, 512], F32R, tag="rtmp")
                        nc.vector.reciprocal(r_tmp[:], ysb[D : D + 1, :])
                        r_bc = ps_y.tile(
                            [128, 512], F32, tag="y",
                            name=f"rbc_{half}_{qb}_{hp}_{hi}",
                        )
                        nc.tensor.matmul(
                            r_bc[:], ones1_sb[:], r_tmp[:], start=True, stop=True
                        )
                        rows = slice(64 * hi, 64 * hi + 64)
                        nc.vector.tensor_mul(
                            y_acc[rows, hp, hoff : hoff + 512],
                            ysb[0:D, :],
                            r_bc[rows, :],
                        )

                def attention_block(half, qb):
                    ktl = _ktiles_for_block(half, qb)
                    pending = []
                    for hp in range(NMH):
                        yus = {}
                        for hi in range(2):
                            yus[hi] = ps_y.tile(
                                [D + 1, 512], F32, tag="y",
                                name=f"yu_{half}_{qb}_{hp}_{hi}",
                            )
                        for ki, (region, j) in enumerate(ktl):
                            qts = _active_qts(half, region, j, qb)
                            qlo, qw = qts[0], len(qts)
                            nq = 128 * qw
                            q_sl = slice(
                                half * T + 128 * qlo, half * T + 128 * (qlo + qw)
                            )
                            rel_sl = slice(
                                128 * (qlo - 4 * qb), 128 * (qlo - 4 * qb + qw)
                            )
                            ktidx = (0 if region == "e1" else 8) + j
                            kbase = (0 if region == "e1" else T) + 128 * j
                            if ki == 1 and pending:
                                _emit_division(*pending.pop(0))
                            for hi in range(2):
                                lo = 64 * hi
                                sp = ps_s.tile([128, 512], F32, tag="s")
                                nc.tensor.matmul(
                                    sp[:, 0:nq],
                                    ktT_sb[lo : lo + 64, hp, kbase : kbase + 128],
                                    qT_sb[lo : lo + 64, hp, q_sl],
                                    start=True,
                                    stop=True,
                                )
                                pt = pt_pool.tile([128, 512], BF16, tag="pt")
                                nc.scalar.activation(
                                    pt[:, 0:nq],
                                    sp[:, 0:nq],
                                    mybir.ActivationFunctionType.Exp,
                                )
                                for qt in qts:
                                    key = (half, region, j, qt)
                                    if key in MASK_IDX:
                                        mi = MASK_IDX[key]
                                        rel = slice(
                                            128 * (qt - qlo), 128 * (qt - qlo + 1)
                                        )
                                        nc.vector.tensor_mul(
                                            pt[:, rel], pt[:, rel], mask_sb[:, mi, :]
                                        )
                                nc.tensor.matmul(
                                    yus[hi][:, rel_sl],
                                    v_sb[:, ktidx, 2 * hp + hi, :],
                                    pt[:, 0:nq],
                                    start=(ki == 0),
                                    stop=(ki == len(ktl) - 1),
                                    skip_group_check=True,
                                )
                        pending.append((half, qb, hp, yus))
                    while pending:
                        _emit_division(*pending.pop(0))

                def partial_proj(half, qb):
                    """Partial output projection (my 512 y-cols) for the 512
                    absolute rows (half, qb); stages bf16 into d_in[qb]."""
                    hoff = half * T + qb * 512
                    mms_extra = []
                    if half == 1:  # proj-LoRA mid partial, e2 rows only
                        tm_ps = ps_misc.tile([RANK, 512], F32, tag="s")
                        _accum(
                            nc,
                            tm_ps[:],
                            [
                                (la_proj_sb[:, hp, :], y_acc[:, hp, hoff : hoff + 512])
                                for hp in range(NMH)
                            ],
                        )
                        tm_sb = small.tile([RANK, 512], F32R, tag="tm")
                        nc.vector.tensor_copy(tm_sb[:], tm_ps[:])
                    for st in range(4):
                        row = hoff + 128 * st
                        for co in range(2):
                            cos = slice(512 * co, 512 * (co + 1))
                            ops = ps_s.tile([128, 512], F32, tag="s")
                            mms = [
                                (y_acc[:, hp, row : row + 128], wproj_sb[:, hp, cos])
                                for hp in range(NMH)
                            ]
                            if half == 1:
                                mms.append(
                                    (
                                        tm_sb[:, 128 * st : 128 * (st + 1)],
                                        lb_proj_sb[:, cos],
                                    )
                                )
                            _accum(nc, ops[:], mms)
                            ost = stage.tile([128, 512], BF16, tag="stage")
                            nc.vector.tensor_copy(ost[:], ops[:])
                            nc.sync.dma_start(
                                out=d_in[qb][half, 128 * st : 128 * (st + 1), cos],
                                in_=ost[:],
                            )

                # ---- schedule: qb0 blocks -> RS0 ; s3 ; qb1 blocks -> RS1 --
                attention_block(0, 0)
                partial_proj(0, 0)
                do_sblock(3)
                attention_block(1, 0)
                partial_proj(1, 0)
                nc.gpsimd.collective_compute(
                    "ReduceScatter",
                    mybir.AluOpType.add,
                    replica_groups=REPLICA_GROUPS,
                    ins=[d_in[0][:]],
                    outs=[d_out[0][:]],
                )
                attention_block(0, 1)
                partial_proj(0, 1)
                attention_block(1, 1)
                partial_proj(1, 1)
                nc.gpsimd.collective_compute(
                    "ReduceScatter",
                    mybir.AluOpType.add,
                    replica_groups=REPLICA_GROUPS,
                    ins=[d_in[1][:]],
                    outs=[d_out[1][:]],
                )

                # ---- receive: bf16 -> f32 convert -> yout -----------------
                for qb in range(2):
                    for st in range(4):
                        row = 512 * qb + 128 * st
                        rsb = rpool.tile([128, C], BF16, tag="recv")
                        nc.sync.dma_start(
                            out=rsb[:],
                            in_=d_out[qb][128 * st : 128 * (st + 1), :],
                        )
                        fsb = rpool.tile([128, C], F32, tag="fcvt")
                        nc.vector.tensor_copy(fsb[:], rsb[:])
                        nc.sync.dma_start(
                            out=yout[row : row + 128, :], in_=fsb[:]
                        )
    return nc


_PROGRAM = None


def _get_program():
    global _PROGRAM
    if _PROGRAM is None:
        _PROGRAM = _build_program()
    return _PROGRAM


# ---------------------------------------------------------------------------
# Host side
# ---------------------------------------------------------------------------
def _delayed_mask_np(t):
    ones = np.ones((t, t), dtype=bool)
    m11 = np.tril(ones) & np.triu(ones, -(LOOKAHEAD + OVERLAP))
    m12 = np.tril(ones, -LOOKAHEAD)
    m21 = np.tril(ones, LOOKAHEAD) & np.triu(ones, -OVERLAP)
    m22 = np.tril(ones)
    return np.block([[m11, m12], [m21, m22]])


_MASKS_CACHE = None


def _packed_masks(M):
    global _MASKS_CACHE
    if _MASKS_CACHE is None:
        mk = np.empty((NMASK, 128, 128), dtype=ml_dtypes.bfloat16)
        for i, (half, region, j, qt) in enumerate(MASK_TILES):
            qg = half * T + 128 * qt
            kg = (0 if region == "e1" else T) + 128 * j
            mk[i] = M[qg : qg + 128, kg : kg + 128].T.astype(np.float32)
        _MASKS_CACHE = mk
    return _MASKS_CACHE


def _core_inputs(core, e1, e2, W_attn, W_proj, la_attn, lb_attn, la_proj, lb_proj, M):
    b, r = core // 2, core % 2
    f32 = np.float32
    bf16 = ml_dtypes.bfloat16
    hs = slice(512 * r, 512 * r + 512)  # my heads' col block

    x = np.concatenate([e1[b], e2[b]], axis=0)  # [2T, C]
    xT = np.ascontiguousarray(x.T).astype(bf16)

    wqk = np.empty((C, 1024), dtype=f32)
    wqk[:, 0:512] = W_attn[:, hs] * QSCALE
    wqk[:, 512:] = W_attn[:, C + 512 * r : C + 512 * r + 512]
    lb_qk = np.empty((RANK, 1024), dtype=f32)
    lb_qk[:, 0:512] = lb_attn[:, hs] * (LSCALE * QSCALE)
    lb_qk[:, 512:] = lb_attn[:, C + 512 * r : C + 512 * r + 512] * LSCALE

    return {
        "xT": xT,
        "wqk": wqk.astype(bf16),
        "wv": np.ascontiguousarray(
            W_attn[:, 2 * C + 512 * r : 2 * C + 512 * r + 512]
        ).astype(bf16),
        "la_attn": np.ascontiguousarray(la_attn).astype(bf16),
        "lb_qk": lb_qk,
        "lb_v": np.ascontiguousarray(
            lb_attn[:, 2 * C + 512 * r : 2 * C + 512 * r + 512], dtype=f32
        )
        * LSCALE,
        "la_proj": np.ascontiguousarray(la_proj[hs, :]).astype(bf16),
        "lb_proj": np.ascontiguousarray(lb_proj, dtype=f32) * LSCALE,
        "wproj": np.ascontiguousarray(W_proj[hs, :]).astype(bf16),
        "masks": _packed_masks(M),
        "ones1": np.ones((1, 128), dtype=f32),
    }


def kernel(
    e1,
    e2,
    W_attn,
    W_proj,
    lora_A_attn,
    lora_B_attn,
    lora_A_proj,
    lora_B_proj,
    _trace=False,
):
    e1 = np.asarray(e1, np.float32)
    e2 = np.asarray(e2, np.float32)
    nc = _get_program()
    M = _delayed_mask_np(T)
    in_maps = [
        _core_inputs(
            c, e1, e2, W_attn, W_proj, lora_A_attn, lora_B_attn, lora_A_proj,
            lora_B_proj, M,
        )
        for c in range(8)
    ]
    res = run_bass_kernel_spmd(nc, in_maps, core_ids=list(range(8)), trace=_trace)
    y1 = np.stack([res.results[2 * b]["yout"] for b in range(B)])
    y2 = np.stack([res.results[2 * b + 1]["yout"] for b in range(B)])
    if _trace:
        kernel.last_results = res
    return y1, y2


# revision 7
# speedup vs baseline: 1.0691x; 1.0691x over previous
"""Trainium2 Bass kernel for nn_DelayedSelfAttention (B=4, T=1024, C=1024, H=16).

Sharding: 8 cores = 4 batches x 2 head-groups.  Core c handles batch c//2
and heads [8r, 8r+8) (r = c%2).  Each core computes Q/K/V for its 8 heads
over the full 2T sequence (no duplicated projection work), attention for
its heads over all 2T query rows, and a PARTIAL output projection for all
2T rows using its heads' 512 columns of y (plus its share of the proj-LoRA
mid).  A pairwise ReduceScatter(add) then sums the two partials and hands
core 2b the e1 rows and core 2b+1 the e2 rows — the collective's rank
order does the role split, so the SPMD program has no role-dependent
addressing at all (masks are identical on every core).

Attention runs in the S^T orientation (keys on partitions, queries on the
free axis): no transposes anywhere.  exp on ScalarE, multiplicative {0,1}
masks on boundary tiles only, AV via V augmented with a ones column so
the softmax denominator accumulates as row 64 of the [65, q] matmul
output.  K^T/V/Q^T stay SBUF-resident (no DRAM spill).  Everything is
bf16 except the f32 PSUM accumulations and the rank-8 LoRA pipes (f32r).
Softmax skips max-subtraction (scores are O(1) by construction).
"""

import contextlib
import sys

for _p in ("/opt/trn_rl_repo", "/root/.axon_site/_ro/trn_rl_repo"):
    if _p not in sys.path:
        sys.path.insert(0, _p)

import ml_dtypes
import numpy as np

import concourse.bass as bass
import concourse.mybir as mybir
import concourse.tile as tile_mod
from concourse.bass_utils import run_bass_kernel_spmd
from concourse.tile import TileContext
from concourse.vector_clock import ScopedClock

# ---------------------------------------------------------------------------
# Workaround: this walrus build supports a single semaphore wait per
# instruction.  Split multi-wait instructions into same-engine NoOps each
# carrying one wait (identical sequencer semantics).
# ---------------------------------------------------------------------------
_ws_counter = [0]


def _fresh_name():
    _ws_counter[0] += 1
    return f"I-waitsplit-{_ws_counter[0]}"


def _split_inst_waits(inst):
    si = inst.sync_info
    if si is None:
        return []
    waits = list(si.on_wait or [])
    if len(waits) <= 1:
        return []
    nops = []
    for w in waits[:-1]:
        nop = mybir.InstNoOp(name=_fresh_name())
        nop.engine = inst.engine
        nop.sync_info = mybir.SyncInfo(on_wait=[w], on_update=[])
        nops.append(nop)
    inst.sync_info = mybir.SyncInfo(
        on_wait=[waits[-1]], on_update=list(si.on_update or [])
    )
    return nops


_orig_lower = tile_mod.TileContext._lower_ordered_insts


def _patched_lower(self, ordered):
    for bb_name in list(ordered.keys()):
        new = []
        for inst in ordered[bb_name]:
            new.extend(_split_inst_waits(inst))
            new.append(inst)
        ordered[bb_name] = new
    return _orig_lower(self, ordered)


def _patched_drain_and_barrier(self, tick_clock, wait_clock):
    nc = self.nc
    drain_inst = nc.sync.drain()
    wait_clock.add_sem_waits(
        drain_inst.ins, ScopedClock({None: tick_clock.global_clock})
    )
    nops = _split_inst_waits(drain_inst.ins)
    if nops:
        first_wait = drain_inst.ins.sync_info
        drain_inst.ins.sync_info = mybir.SyncInfo(on_wait=[], on_update=[])
        for nop in nops:
            n2 = nc.sync.nop(nofuse=True)
            n2.ins.sync_info = nop.sync_info
        d2 = nc.sync.drain()
        d2.ins.sync_info = first_wait

    nc.all_engine_barrier()
    assert self.sems is not None
    popped = nc._tile_sem_poison_stack.pop()
    assert popped is self._sem_poison
    nc.clear_and_free_semaphores(list(self.sems.allocated().values()))
    nc.all_engine_barrier()


def _apply_tile_patch():
    if tile_mod.TileContext._lower_ordered_insts is not _patched_lower:
        tile_mod.TileContext._lower_ordered_insts = _patched_lower
        tile_mod.TileContext._drain_and_barrier = _patched_drain_and_barrier


# ---------------------------------------------------------------------------
# Problem constants (hardcoded per the task contract).
# ---------------------------------------------------------------------------
B, T, C, H = 4, 1024, 1024, 16
D = C // H  # 64
SEQ = 2 * T
LOOKAHEAD, OVERLAP = 64, 64
RANK, ALPHA = 8, 16.0
LSCALE = ALPHA / RANK  # 2.0
QSCALE = 1.0 / np.sqrt(D)  # 1/8
NCH = C // 128  # 8 contraction chunks
NMH = 4  # my-head 128-chunks (512 cols / 128)
F32 = mybir.dt.float32
F32R = mybir.dt.float32r
BF16 = mybir.dt.bfloat16

REPLICA_GROUPS = [[0, 1], [2, 3], [4, 5], [6, 7]]


# Trace-time tiling structure, shared by host (mask packing) and device.
# half: 0 = e1 queries, 1 = e2 queries.  qt/j are local 128-tiles (0..7).
def _ktiles_for_block(half, qb):
    """k-tiles (region, j) touched by q-subtiles [4qb, 4qb+4) of `half`."""
    qts = range(4 * qb, 4 * qb + 4)
    if half == 0:
        e1 = sorted({j for qt in qts for j in (qt - 1, qt) if 0 <= j < 8})
    else:
        e1 = sorted({j for qt in qts for j in (qt - 1, qt, qt + 1) if 0 <= j < 8})
    e2 = sorted({j for qt in qts for j in range(qt + 1)})
    return [("e1", j) for j in e1] + [("e2", j) for j in e2]


def _active_qts(half, region, j, qb):
    if region == "e1":
        cand = {j, j + 1} if half == 0 else {j - 1, j, j + 1}
    else:
        cand = set(range(j, 8))
    qts = sorted(cand & set(range(4 * qb, 4 * qb + 4)))
    assert qts == list(range(qts[0], qts[-1] + 1))
    return qts


def _mask_tiles():
    out = []
    for half in (0, 1):
        for qt in range(8):
            if half == 0:
                for j in (qt - 1, qt):
                    if 0 <= j < 8:
                        out.append((half, "e1", j, qt))
                for j in (qt - 1, qt):
                    if j >= 0:
                        out.append((half, "e2", j, qt))
            else:
                for j in (qt - 1, qt, qt + 1):
                    if 0 <= j < 8:
                        out.append((half, "e1", j, qt))
                out.append((half, "e2", qt, qt))
    return out


MASK_TILES = _mask_tiles()  # 60 tiles
MASK_IDX = {k: i for i, k in enumerate(MASK_TILES)}
NMASK = len(MASK_TILES)


def _accum(nc, out_ps, pairs):
    """Accumulating matmul group: list of (lhsT, rhs) into one psum tile."""
    n = len(pairs)
    for i, (lh, rh) in enumerate(pairs):
        nc.tensor.matmul(out_ps, lh, rh, start=(i == 0), stop=(i == n - 1))


# ---------------------------------------------------------------------------
# Device program (identical on all 8 cores; role differences live in data
# and in the ReduceScatter rank order).
# ---------------------------------------------------------------------------
def _build_program():
    _apply_tile_patch()
    nc = bass.Bass("TRN2", target_bir_lowering=False, debug=False, num_devices=8)

    def din(name, shape, dt=F32R):
        return nc.dram_tensor(name, list(shape), dt, kind="ExternalInput").ap()

    xT = din("xT", (C, SEQ), dt=BF16)
    wqk = din("wqk", (C, 1024), dt=BF16)  # [q my512 (prescaled 1/8) | k my512]
    wv = din("wv", (C, 512), dt=BF16)
    la_attn = din("la_attn", (C, RANK), dt=BF16)
    lb_qk = din("lb_qk", (RANK, 1024))  # scaled; q part also 1/8
    lb_v = din("lb_v", (RANK, 512))
    la_proj = din("la_proj", (512, RANK), dt=BF16)  # rows = my 512 y-cols
    lb_proj = din("lb_proj", (RANK, C))
    wproj = din("wproj", (512, C), dt=BF16)  # rows = my 512 y-cols
    masks = din("masks", (NMASK, 128, 128), dt=BF16)
    ones1 = din("ones1", (1, 128))
    yout = nc.dram_tensor("yout", [T, C], BF16, kind="ExternalOutput").ap()

    with TileContext(nc) as tc:
        ctx = contextlib.ExitStack()
        with ctx:
            ctx.enter_context(
                nc.allow_low_precision(reason="float32r is full-width fp32 storage")
            )
            # DRAM staging for the pairwise partial-output ReduceScatter.
            dpool = ctx.enter_context(tc.tile_pool(name="dram", bufs=1, space="DRAM"))
            d_in = [
                dpool.tile([2, 512, C], BF16, name=f"d_in{k}") for k in range(2)
            ]
            d_out = [
                dpool.tile([512, C], BF16, name=f"d_out{k}") for k in range(2)
            ]

            # --- persistent SBUF ---
            persist = ctx.enter_context(tc.tile_pool(name="persist", bufs=1))
            qT_sb = persist.tile([128, NMH, SEQ], BF16)
            ktT_sb = persist.tile([128, NMH, SEQ], BF16)
            v_sb = persist.tile([128, 16, 8, D + 1], BF16)  # (part, kt, head, d+1)
            y_acc = persist.tile([128, NMH, SEQ], BF16)  # y^T, my heads
            la_attn_sb = persist.tile([128, NCH, RANK], BF16)
            lb_qk_sb = persist.tile([RANK, 1024], F32R)
            lb_v_sb = persist.tile([RANK, 512], F32R)
            la_proj_sb = persist.tile([128, NMH, RANK], BF16)
            lb_proj_sb = persist.tile([RANK, C], F32R)
            ones1_sb = persist.tile([1, 128], F32R)
            mask_sb = persist.tile([128, NMASK, 128], BF16)
            tmp_kv_sb = persist.tile([RANK, T], F32R)  # attn-lora mid, e2 rows

            nc.sync.dma_start(
                out=la_attn_sb[:], in_=la_attn.rearrange("(ch p) r -> p ch r", p=128)
            )
            nc.sync.dma_start(out=lb_qk_sb[:], in_=lb_qk[:])
            nc.sync.dma_start(out=lb_v_sb[:], in_=lb_v[:])
            nc.sync.dma_start(
                out=la_proj_sb[:], in_=la_proj.rearrange("(ch p) r -> p ch r", p=128)
            )
            nc.sync.dma_start(out=lb_proj_sb[:], in_=lb_proj[:])
            nc.sync.dma_start(out=ones1_sb[:], in_=ones1[:])

            # Ones column of the augmented V (denominator accumulator rows).
            nc.gpsimd.memset(v_sb[:, :, :, D : D + 1], 1.0)

            # --- PSUM pools ---
            ps_s = ctx.enter_context(tc.tile_pool(name="ps_s", bufs=4, space="PSUM"))
            ps_y = ctx.enter_context(tc.tile_pool(name="ps_y", bufs=4, space="PSUM"))
            ps_misc = ps_s

            stage = ctx.enter_context(tc.tile_pool(name="stage", bufs=4))
            rpool = ctx.enter_context(tc.tile_pool(name="rpool", bufs=2))
            small = ctx.enter_context(tc.tile_pool(name="small", bufs=3))
            pt_pool = ctx.enter_context(tc.tile_pool(name="pt", bufs=6))

            # ====== Phase A: Q^T / K^T / V for my heads, full 2T, resident ==
            wqk_ctx = tc.tile_pool(name="wqk_pool", bufs=1)
            xa_ctx = tc.tile_pool(name="xa", bufs=2)
            with wqk_ctx as wqk_pool, xa_ctx as xa_pool:
                xa_first = xa_pool.tile([128, NCH, 512], BF16, tag="xa", name="xa_first")
                wqk_sb = wqk_pool.tile([128, NCH, 1024], BF16)
                wv_sb = wqk_pool.tile([128, NCH, 512], BF16)
                for ch in range(NCH):
                    nc.sync.dma_start(
                        out=wqk_sb[:, ch, :],
                        in_=wqk[128 * ch : 128 * (ch + 1), :],
                    )
                    nc.sync.dma_start(
                        out=xa_first[:, ch, :],
                        in_=xT[128 * ch : 128 * (ch + 1), 0:512],
                    )
                for ch in range(NCH):
                    nc.sync.dma_start(
                        out=wv_sb[:, ch, :],
                        in_=wv[128 * ch : 128 * (ch + 1), :],
                    )

                def do_sblock(s):
                    sl = slice(s * 512, (s + 1) * 512)
                    if s == 0:
                        xt_s = xa_first
                    else:
                        xt_s = xa_pool.tile([128, NCH, 512], BF16, tag="xa")
                        for ch in range(NCH):
                            nc.sync.dma_start(
                                out=xt_s[:, ch, :],
                                in_=xT[128 * ch : 128 * (ch + 1), sl],
                            )
                    tsl = None
                    if s >= 2:  # e2 rows: attn-lora mid  tmp^T = A^T x
                        tsl = slice((s - 2) * 512, (s - 1) * 512)
                        tmp_ps = ps_misc.tile([RANK, 512], F32, tag="s")
                        _accum(
                            nc,
                            tmp_ps[:],
                            [
                                (la_attn_sb[:, ch, :], xt_s[:, ch, :])
                                for ch in range(NCH)
                            ],
                        )
                        nc.vector.tensor_copy(tmp_kv_sb[:, tsl], tmp_ps[:])
                    for m in range(NMH):  # K^T cols (my heads)
                        cols = slice(512 + 128 * m, 512 + 128 * (m + 1))
                        kps = ps_s.tile([128, 512], F32, tag="s")
                        mms = [
                            (wqk_sb[:, ch, cols], xt_s[:, ch, :]) for ch in range(NCH)
                        ]
                        if s >= 2:
                            mms.append((lb_qk_sb[:, cols], tmp_kv_sb[:, tsl]))
                        _accum(nc, kps[:], mms)
                        nc.vector.tensor_copy(ktT_sb[:, m, sl], kps[:])
                    for m in range(NMH):  # Q^T (prescaled by 1/8 via wqk/lb data)
                        cols = slice(128 * m, 128 * (m + 1))
                        qps = ps_s.tile([128, 512], F32, tag="s")
                        mms = [
                            (wqk_sb[:, ch, cols], xt_s[:, ch, :]) for ch in range(NCH)
                        ]
                        if s >= 2:
                            mms.append((lb_qk_sb[:, cols], tmp_kv_sb[:, tsl]))
                        _accum(nc, qps[:], mms)
                        nc.vector.tensor_copy(qT_sb[:, m, sl], qps[:])
                    for st in range(4):  # V rows (128-row seq tiles)
                        vps = ps_s.tile([128, 512], F32, tag="s")
                        mms = [
                            (
                                xt_s[:, ch, 128 * st : 128 * (st + 1)],
                                wv_sb[:, ch, :],
                            )
                            for ch in range(NCH)
                        ]
                        if s >= 2:
                            base = (s - 2) * 512 + 128 * st
                            mms.append((tmp_kv_sb[:, base : base + 128], lb_v_sb[:]))
                        _accum(nc, vps[:], mms)
                        nc.vector.tensor_copy(
                            v_sb[:, 4 * s + st, :, 0:D],
                            vps[:].rearrange("p (h d) -> p h d", h=8),
                        )

                for s in (0, 2):
                    do_sblock(s)
                nc.sync.dma_start(
                    out=mask_sb[:], in_=masks.rearrange("t p q -> p t q")
                )

                # ===== Phase B/C helpers =====
                wproj_sb = persist.tile([128, NMH, C], BF16)
                for m in range(NMH):
                    nc.sync.dma_start(
                        out=wproj_sb[:, m, :],
                        in_=wproj[128 * m : 128 * (m + 1), :],
                    )

                def _emit_division(half, qb, hp, yus):
                    hoff = half * T + qb * 512
                    for hi in range(2):
                        yu = yus[hi]
                        ysb = pt_pool.tile(
                            [D + 1, 512], F32R, tag="ysb",
                            name=f"ysb_{half}_{qb}_{hp}_{hi}",
                        )
                        nc.vector.tensor_copy(ysb[:], yu[:])
                        r_tmp = small.tile([1, 512], F32R, tag="rtmp")
                        nc.vector.reciprocal(r_tmp[:], ysb[D : D + 1, :])
                        r_bc = ps_s.tile(
                            [128, 512], F32, tag="s",
                            name=f"rbc_{half}_{qb}_{hp}_{hi}",
                        )
                        nc.tensor.matmul(
                            r_bc[:], ones1_sb[:], r_tmp[:], start=True, stop=True
                        )
                        rows = slice(64 * hi, 64 * hi + 64)
                        nc.vector.tensor_mul(
                            y_acc[rows, hp, hoff : hoff + 512],
                            ysb[0:D, :],
                            r_bc[rows, :],
                        )

                def attention_block(half, qb):
                    ktl = _ktiles_for_block(half, qb)
                    nk = len(ktl)
                    pending = []
                    for hp in range(NMH):
                        yus = {}
                        for hi in range(2):
                            yus[hi] = ps_y.tile(
                                [D + 1, 512], F32, tag="y",
                                name=f"yu_{half}_{qb}_{hp}_{hi}",
                            )
                        # 1-deep software pipeline: scores/exp for ktile ki are
                        # emitted before the AV matmuls of ktile ki-1 so the PE
                        # never sits behind an exp it just enabled.
                        prev = None
                        for ki in range(nk + 1):
                            cur = None
                            if ki < nk:
                                region, j = ktl[ki]
                                qts = _active_qts(half, region, j, qb)
                                qlo, qw = qts[0], len(qts)
                                nq = 128 * qw
                                q_sl = slice(
                                    half * T + 128 * qlo,
                                    half * T + 128 * (qlo + qw),
                                )
                                rel_sl = slice(
                                    128 * (qlo - 4 * qb), 128 * (qlo - 4 * qb + qw)
                                )
                                ktidx = (0 if region == "e1" else 8) + j
                                kbase = (0 if region == "e1" else T) + 128 * j
                                pts = []
                                for hi in range(2):
                                    lo = 64 * hi
                                    sp = ps_s.tile([128, 512], F32, tag="s")
                                    nc.tensor.matmul(
                                        sp[:, 0:nq],
                                        ktT_sb[lo : lo + 64, hp, kbase : kbase + 128],
                                        qT_sb[lo : lo + 64, hp, q_sl],
                                        start=True,
                                        stop=True,
                                    )
                                    pt = pt_pool.tile([128, 512], BF16, tag="pt")
                                    nc.scalar.activation(
                                        pt[:, 0:nq],
                                        sp[:, 0:nq],
                                        mybir.ActivationFunctionType.Exp,
                                    )
                                    for qt in qts:
                                        key = (half, region, j, qt)
                                        if key in MASK_IDX:
                                            mi = MASK_IDX[key]
                                            rel = slice(
                                                128 * (qt - qlo),
                                                128 * (qt - qlo + 1),
                                            )
                                            nc.vector.tensor_mul(
                                                pt[:, rel],
                                                pt[:, rel],
                                                mask_sb[:, mi, :],
                                            )
                                    pts.append(pt)
                                cur = (ki, ktidx, rel_sl, nq, pts)
                                if ki == 1 and pending:
                                    _emit_division(*pending.pop(0))
                            if prev is not None:
                                pki, pktidx, prel, pnq, ppts = prev
                                for hi in range(2):
                                    nc.tensor.matmul(
                                        yus[hi][:, prel],
                                        v_sb[:, pktidx, 2 * hp + hi, :],
                                        ppts[hi][:, 0:pnq],
                                        start=(pki == 0),
                                        stop=(pki == nk - 1),
                                        skip_group_check=True,
                                    )
                            prev = cur
                        pending.append((half, qb, hp, yus))
                    while pending:
                        _emit_division(*pending.pop(0))

                def partial_proj(half, qb):
                    """Partial output projection (my 512 y-cols) for the 512
                    absolute rows (half, qb); stages bf16 into d_in[qb]."""
                    hoff = half * T + qb * 512
                    mms_extra = []
                    if half == 1:  # proj-LoRA mid partial, e2 rows only
                        tm_ps = ps_misc.tile([RANK, 512], F32, tag="s")
                        _accum(
                            nc,
                            tm_ps[:],
                            [
                                (la_proj_sb[:, hp, :], y_acc[:, hp, hoff : hoff + 512])
                                for hp in range(NMH)
                            ],
                        )
                        tm_sb = small.tile([RANK, 512], F32R, tag="tm")
                        nc.vector.tensor_copy(tm_sb[:], tm_ps[:])
                    for st in range(4):
                        row = hoff + 128 * st
                        for co in range(2):
                            cos = slice(512 * co, 512 * (co + 1))
                            ops = ps_s.tile([128, 512], F32, tag="s")
                            mms = [
                                (y_acc[:, hp, row : row + 128], wproj_sb[:, hp, cos])
                                for hp in range(NMH)
                            ]
                            if half == 1:
                                mms.append(
                                    (
                                        tm_sb[:, 128 * st : 128 * (st + 1)],
                                        lb_proj_sb[:, cos],
                                    )
                                )
                            _accum(nc, ops[:], mms)
                            ost = stage.tile([128, 512], BF16, tag="stage")
                            nc.vector.tensor_copy(ost[:], ops[:])
                            nc.sync.dma_start(
                                out=d_in[qb][half, 128 * st : 128 * (st + 1), cos],
                                in_=ost[:],
                            )

                # ---- schedule: interleave attention with remaining
                # s-blocks; RS0 overlaps s3 + qb1 attention ------------------
                attention_block(0, 0)
                partial_proj(0, 0)
                do_sblock(1)
                attention_block(1, 0)
                partial_proj(1, 0)
                nc.gpsimd.collective_compute(
                    "ReduceScatter",
                    mybir.AluOpType.add,
                    replica_groups=REPLICA_GROUPS,
                    ins=[d_in[0][:]],
                    outs=[d_out[0][:]],
                )
                do_sblock(3)
                attention_block(0, 1)
                partial_proj(0, 1)
                attention_block(1, 1)
                partial_proj(1, 1)
                nc.gpsimd.collective_compute(
                    "ReduceScatter",
                    mybir.AluOpType.add,
                    replica_groups=REPLICA_GROUPS,
                    ins=[d_in[1][:]],
                    outs=[d_out[1][:]],
                )

                # ---- receive: direct DRAM->DRAM into the bf16 output ------
                for qb in range(2):
                    nc.sync.dma_start(
                        out=yout[512 * qb : 512 * (qb + 1), :], in_=d_out[qb][:]
                    )
    return nc


_PROGRAM = None


def _get_program():
    global _PROGRAM
    if _PROGRAM is None:
        _PROGRAM = _build_program()
    return _PROGRAM


# ---------------------------------------------------------------------------
# Host side
# ---------------------------------------------------------------------------
def _delayed_mask_np(t):
    ones = np.ones((t, t), dtype=bool)
    m11 = np.tril(ones) & np.triu(ones, -(LOOKAHEAD + OVERLAP))
    m12 = np.tril(ones, -LOOKAHEAD)
    m21 = np.tril(ones, LOOKAHEAD) & np.triu(ones, -OVERLAP)
    m22 = np.tril(ones)
    return np.block([[m11, m12], [m21, m22]])


_MASKS_CACHE = None


def _packed_masks(M):
    global _MASKS_CACHE
    if _MASKS_CACHE is None:
        mk = np.empty((NMASK, 128, 128), dtype=ml_dtypes.bfloat16)
        for i, (half, region, j, qt) in enumerate(MASK_TILES):
            qg = half * T + 128 * qt
            kg = (0 if region == "e1" else T) + 128 * j
            mk[i] = M[qg : qg + 128, kg : kg + 128].T.astype(np.float32)
        _MASKS_CACHE = mk
    return _MASKS_CACHE


def _core_inputs(core, e1, e2, W_attn, W_proj, la_attn, lb_attn, la_proj, lb_proj, M):
    b, r = core // 2, core % 2
    f32 = np.float32
    bf16 = ml_dtypes.bfloat16
    hs = slice(512 * r, 512 * r + 512)  # my heads' col block

    x = np.concatenate([e1[b], e2[b]], axis=0)  # [2T, C]
    xT = np.ascontiguousarray(x.T).astype(bf16)

    wqk = np.empty((C, 1024), dtype=f32)
    wqk[:, 0:512] = W_attn[:, hs] * QSCALE
    wqk[:, 512:] = W_attn[:, C + 512 * r : C + 512 * r + 512]
    lb_qk = np.empty((RANK, 1024), dtype=f32)
    lb_qk[:, 0:512] = lb_attn[:, hs] * (LSCALE * QSCALE)
    lb_qk[:, 512:] = lb_attn[:, C + 512 * r : C + 512 * r + 512] * LSCALE

    return {
        "xT": xT,
        "wqk": wqk.astype(bf16),
        "wv": np.ascontiguousarray(
            W_attn[:, 2 * C + 512 * r : 2 * C + 512 * r + 512]
        ).astype(bf16),
        "la_attn": np.ascontiguousarray(la_attn).astype(bf16),
        "lb_qk": lb_qk,
        "lb_v": np.ascontiguousarray(
            lb_attn[:, 2 * C + 512 * r : 2 * C + 512 * r + 512], dtype=f32
        )
        * LSCALE,
        "la_proj": np.ascontiguousarray(la_proj[hs, :]).astype(bf16),
        "lb_proj": np.ascontiguousarray(lb_proj, dtype=f32) * LSCALE,
        "wproj": np.ascontiguousarray(W_proj[hs, :]).astype(bf16),
        "masks": _packed_masks(M),
        "ones1": np.ones((1, 128), dtype=f32),
    }


def kernel(
    e1,
    e2,
    W_attn,
    W_proj,
    lora_A_attn,
    lora_B_attn,
    lora_A_proj,
    lora_B_proj,
    _trace=False,
):
    e1 = np.asarray(e1, np.float32)
    e2 = np.asarray(e2, np.float32)
    nc = _get_program()
    M = _delayed_mask_np(T)
    in_maps = [
        _core_inputs(
            c, e1, e2, W_attn, W_proj, lora_A_attn, lora_B_attn, lora_A_proj,
            lora_B_proj, M,
        )
        for c in range(8)
    ]
    res = run_bass_kernel_spmd(nc, in_maps, core_ids=list(range(8)), trace=_trace)
    y1 = np.stack([res.results[2 * b]["yout"] for b in range(B)])
    y2 = np.stack([res.results[2 * b + 1]["yout"] for b in range(B)])
    if _trace:
        kernel.last_results = res
    return y1, y2


# revision 8
# speedup vs baseline: 1.1348x; 1.0614x over previous
"""Trainium2 Bass kernel for nn_DelayedSelfAttention (B=4, T=1024, C=1024, H=16).

Sharding: 8 cores = 4 batches x 2 head-groups.  Core c handles batch c//2
and heads [8r, 8r+8) (r = c%2).  Each core computes Q/K/V for its 8 heads
over the full 2T sequence (no duplicated projection work), attention for
its heads over all 2T query rows, and a PARTIAL output projection for all
2T rows using its heads' 512 columns of y (plus its share of the proj-LoRA
mid).  A pairwise ReduceScatter(add) then sums the two partials and hands
core 2b the e1 rows and core 2b+1 the e2 rows — the collective's rank
order does the role split, so the SPMD program has no role-dependent
addressing at all (masks are identical on every core).

Attention runs in the S^T orientation (keys on partitions, queries on the
free axis): no transposes anywhere.  exp on ScalarE, multiplicative {0,1}
masks on boundary tiles only, AV via V augmented with a ones column so
the softmax denominator accumulates as row 64 of the [65, q] matmul
output.  K^T/V/Q^T stay SBUF-resident (no DRAM spill).  Everything is
bf16 except the f32 PSUM accumulations and the rank-8 LoRA pipes (f32r).
Softmax skips max-subtraction (scores are O(1) by construction).
"""

import contextlib
import sys

for _p in ("/opt/trn_rl_repo", "/root/.axon_site/_ro/trn_rl_repo"):
    if _p not in sys.path:
        sys.path.insert(0, _p)

import ml_dtypes
import numpy as np

import concourse.bass as bass
import concourse.mybir as mybir
import concourse.tile as tile_mod
from concourse.bass_utils import run_bass_kernel_spmd
from concourse.tile import TileContext
from concourse.vector_clock import ScopedClock

# ---------------------------------------------------------------------------
# Workaround: this walrus build supports a single semaphore wait per
# instruction.  Split multi-wait instructions into same-engine NoOps each
# carrying one wait (identical sequencer semantics).
# ---------------------------------------------------------------------------
_ws_counter = [0]


def _fresh_name():
    _ws_counter[0] += 1
    return f"I-waitsplit-{_ws_counter[0]}"


def _split_inst_waits(inst):
    si = inst.sync_info
    if si is None:
        return []
    waits = list(si.on_wait or [])
    if len(waits) <= 1:
        return []
    nops = []
    for w in waits[:-1]:
        nop = mybir.InstNoOp(name=_fresh_name())
        nop.engine = inst.engine
        nop.sync_info = mybir.SyncInfo(on_wait=[w], on_update=[])
        nops.append(nop)
    inst.sync_info = mybir.SyncInfo(
        on_wait=[waits[-1]], on_update=list(si.on_update or [])
    )
    return nops


_orig_lower = tile_mod.TileContext._lower_ordered_insts


def _patched_lower(self, ordered):
    for bb_name in list(ordered.keys()):
        new = []
        for inst in ordered[bb_name]:
            new.extend(_split_inst_waits(inst))
            new.append(inst)
        ordered[bb_name] = new
    return _orig_lower(self, ordered)


def _patched_drain_and_barrier(self, tick_clock, wait_clock):
    nc = self.nc
    drain_inst = nc.sync.drain()
    wait_clock.add_sem_waits(
        drain_inst.ins, ScopedClock({None: tick_clock.global_clock})
    )
    nops = _split_inst_waits(drain_inst.ins)
    if nops:
        first_wait = drain_inst.ins.sync_info
        drain_inst.ins.sync_info = mybir.SyncInfo(on_wait=[], on_update=[])
        for nop in nops:
            n2 = nc.sync.nop(nofuse=True)
            n2.ins.sync_info = nop.sync_info
        d2 = nc.sync.drain()
        d2.ins.sync_info = first_wait

    nc.all_engine_barrier()
    assert self.sems is not None
    popped = nc._tile_sem_poison_stack.pop()
    assert popped is self._sem_poison
    nc.clear_and_free_semaphores(list(self.sems.allocated().values()))
    nc.all_engine_barrier()


def _apply_tile_patch():
    if tile_mod.TileContext._lower_ordered_insts is not _patched_lower:
        tile_mod.TileContext._lower_ordered_insts = _patched_lower
        tile_mod.TileContext._drain_and_barrier = _patched_drain_and_barrier


# ---------------------------------------------------------------------------
# Problem constants (hardcoded per the task contract).
# ---------------------------------------------------------------------------
B, T, C, H = 4, 1024, 1024, 16
D = C // H  # 64
SEQ = 2 * T
LOOKAHEAD, OVERLAP = 64, 64
RANK, ALPHA = 8, 16.0
LSCALE = ALPHA / RANK  # 2.0
QSCALE = 1.0 / np.sqrt(D)  # 1/8
NCH = C // 128  # 8 contraction chunks
NMH = 4  # my-head 128-chunks (512 cols / 128)
F32 = mybir.dt.float32
F32R = mybir.dt.float32r
BF16 = mybir.dt.bfloat16

REPLICA_GROUPS = [[0, 1], [2, 3], [4, 5], [6, 7]]


# Trace-time tiling structure, shared by host (mask packing) and device.
# half: 0 = e1 queries, 1 = e2 queries.  qt/j are local 128-tiles (0..7).
def _ktiles_for_block(half, qb):
    """k-tiles (region, j) touched by q-subtiles [4qb, 4qb+4) of `half`."""
    qts = range(4 * qb, 4 * qb + 4)
    if half == 0:
        e1 = sorted({j for qt in qts for j in (qt - 1, qt) if 0 <= j < 8})
    else:
        e1 = sorted({j for qt in qts for j in (qt - 1, qt, qt + 1) if 0 <= j < 8})
    e2 = sorted({j for qt in qts for j in range(qt + 1)})
    return [("e1", j) for j in e1] + [("e2", j) for j in e2]


def _active_qts(half, region, j, qb):
    if region == "e1":
        cand = {j, j + 1} if half == 0 else {j - 1, j, j + 1}
    else:
        cand = set(range(j, 8))
    qts = sorted(cand & set(range(4 * qb, 4 * qb + 4)))
    assert qts == list(range(qts[0], qts[-1] + 1))
    return qts


def _mask_tiles():
    out = []
    for half in (0, 1):
        for qt in range(8):
            if half == 0:
                for j in (qt - 1, qt):
                    if 0 <= j < 8:
                        out.append((half, "e1", j, qt))
                for j in (qt - 1, qt):
                    if j >= 0:
                        out.append((half, "e2", j, qt))
            else:
                for j in (qt - 1, qt, qt + 1):
                    if 0 <= j < 8:
                        out.append((half, "e1", j, qt))
                out.append((half, "e2", qt, qt))
    return out


MASK_TILES = _mask_tiles()  # 60 tiles
MASK_IDX = {k: i for i, k in enumerate(MASK_TILES)}
NMASK = len(MASK_TILES)


def _accum(nc, out_ps, pairs):
    """Accumulating matmul group: list of (lhsT, rhs) into one psum tile."""
    n = len(pairs)
    for i, (lh, rh) in enumerate(pairs):
        nc.tensor.matmul(out_ps, lh, rh, start=(i == 0), stop=(i == n - 1))


# ---------------------------------------------------------------------------
# Device program (identical on all 8 cores; role differences live in data
# and in the ReduceScatter rank order).
# ---------------------------------------------------------------------------
def _build_program():
    _apply_tile_patch()
    nc = bass.Bass("TRN2", target_bir_lowering=False, debug=False, num_devices=8)

    def din(name, shape, dt=F32R):
        return nc.dram_tensor(name, list(shape), dt, kind="ExternalInput").ap()

    xT = din("xT", (C, SEQ), dt=BF16)
    wqk = din("wqk", (C, 1024), dt=BF16)  # [q my512 (prescaled 1/8) | k my512]
    wv = din("wv", (C, 512), dt=BF16)
    la_attn = din("la_attn", (C, RANK), dt=BF16)
    lb_qk = din("lb_qk", (RANK, 1024))  # scaled; q part also 1/8
    lb_v = din("lb_v", (RANK, 512))
    la_proj = din("la_proj", (512, RANK), dt=BF16)  # rows = my 512 y-cols
    lb_proj = din("lb_proj", (RANK, C))
    wproj = din("wproj", (512, C), dt=BF16)  # rows = my 512 y-cols
    masks = din("masks", (NMASK, 128, 128), dt=BF16)  # 0 / -40 penalties
    iden = din("iden", (128, 128), dt=BF16)
    ones1 = din("ones1", (1, 128))
    yout = nc.dram_tensor("yout", [T, C], BF16, kind="ExternalOutput").ap()

    with TileContext(nc) as tc:
        ctx = contextlib.ExitStack()
        with ctx:
            ctx.enter_context(
                nc.allow_low_precision(reason="float32r is full-width fp32 storage")
            )
            # DRAM staging for the pairwise partial-output ReduceScatter.
            dpool = ctx.enter_context(tc.tile_pool(name="dram", bufs=1, space="DRAM"))
            d_in = [
                dpool.tile([2, 512, C], BF16, name=f"d_in{k}") for k in range(2)
            ]
            d_out = [
                dpool.tile([512, C], BF16, name=f"d_out{k}") for k in range(2)
            ]

            # --- persistent SBUF ---
            persist = ctx.enter_context(tc.tile_pool(name="persist", bufs=1))
            qT_sb = persist.tile([128, NMH, SEQ], BF16)
            ktT_sb = persist.tile([128, NMH, SEQ], BF16)
            v_sb = persist.tile([128, 16, 8, D + 1], BF16)  # (part, kt, head, d+1)
            y_acc = persist.tile([128, NMH, SEQ], BF16)  # y^T, my heads
            la_attn_sb = persist.tile([128, NCH, RANK], BF16)
            lb_qk_sb = persist.tile([RANK, 1024], F32R)
            lb_v_sb = persist.tile([RANK, 512], F32R)
            la_proj_sb = persist.tile([128, NMH, RANK], BF16)
            lb_proj_sb = persist.tile([RANK, C], F32R)
            ones1_sb = persist.tile([1, 128], F32R)
            iden_sb = persist.tile([128, 128], BF16)
            mask_sb = persist.tile([128, NMASK, 128], BF16)
            tmp_kv_sb = persist.tile([RANK, T], F32R)  # attn-lora mid, e2 rows

            nc.sync.dma_start(
                out=la_attn_sb[:], in_=la_attn.rearrange("(ch p) r -> p ch r", p=128)
            )
            nc.sync.dma_start(out=lb_qk_sb[:], in_=lb_qk[:])
            nc.sync.dma_start(out=lb_v_sb[:], in_=lb_v[:])
            nc.sync.dma_start(
                out=la_proj_sb[:], in_=la_proj.rearrange("(ch p) r -> p ch r", p=128)
            )
            nc.sync.dma_start(out=lb_proj_sb[:], in_=lb_proj[:])
            nc.sync.dma_start(out=ones1_sb[:], in_=ones1[:])
            nc.sync.dma_start(out=iden_sb[:], in_=iden[:])

            # Ones column of the augmented V (denominator accumulator rows).
            nc.gpsimd.memset(v_sb[:, :, :, D : D + 1], 1.0)

            # --- PSUM pools ---
            ps_s = ctx.enter_context(tc.tile_pool(name="ps_s", bufs=4, space="PSUM"))
            ps_y = ctx.enter_context(tc.tile_pool(name="ps_y", bufs=4, space="PSUM"))
            ps_misc = ps_s

            stage = ctx.enter_context(tc.tile_pool(name="stage", bufs=4))
            rpool = ctx.enter_context(tc.tile_pool(name="rpool", bufs=2))
            small = ctx.enter_context(tc.tile_pool(name="small", bufs=3))
            pt_pool = ctx.enter_context(tc.tile_pool(name="pt", bufs=6))

            # ====== Phase A: Q^T / K^T / V for my heads, full 2T, resident ==
            wqk_ctx = tc.tile_pool(name="wqk_pool", bufs=1)
            xa_ctx = tc.tile_pool(name="xa", bufs=2)
            with wqk_ctx as wqk_pool, xa_ctx as xa_pool:
                xa_first = xa_pool.tile([128, NCH, 512], BF16, tag="xa", name="xa_first")
                wqk_sb = wqk_pool.tile([128, NCH, 1024], BF16)
                wv_sb = wqk_pool.tile([128, NCH, 512], BF16)
                for ch in range(NCH):
                    nc.sync.dma_start(
                        out=wqk_sb[:, ch, :],
                        in_=wqk[128 * ch : 128 * (ch + 1), :],
                    )
                    nc.sync.dma_start(
                        out=xa_first[:, ch, :],
                        in_=xT[128 * ch : 128 * (ch + 1), 0:512],
                    )
                for ch in range(NCH):
                    nc.sync.dma_start(
                        out=wv_sb[:, ch, :],
                        in_=wv[128 * ch : 128 * (ch + 1), :],
                    )

                def do_sblock(s):
                    sl = slice(s * 512, (s + 1) * 512)
                    if s == 0:
                        xt_s = xa_first
                    else:
                        xt_s = xa_pool.tile([128, NCH, 512], BF16, tag="xa")
                        for ch in range(NCH):
                            nc.sync.dma_start(
                                out=xt_s[:, ch, :],
                                in_=xT[128 * ch : 128 * (ch + 1), sl],
                            )
                    tsl = None
                    if s >= 2:  # e2 rows: attn-lora mid  tmp^T = A^T x
                        tsl = slice((s - 2) * 512, (s - 1) * 512)
                        tmp_ps = ps_misc.tile([RANK, 512], F32, tag="s")
                        _accum(
                            nc,
                            tmp_ps[:],
                            [
                                (la_attn_sb[:, ch, :], xt_s[:, ch, :])
                                for ch in range(NCH)
                            ],
                        )
                        nc.vector.tensor_copy(tmp_kv_sb[:, tsl], tmp_ps[:])
                    for m in range(NMH):  # K^T cols (my heads)
                        cols = slice(512 + 128 * m, 512 + 128 * (m + 1))
                        kps = ps_s.tile([128, 512], F32, tag="s")
                        mms = [
                            (wqk_sb[:, ch, cols], xt_s[:, ch, :]) for ch in range(NCH)
                        ]
                        if s >= 2:
                            mms.append((lb_qk_sb[:, cols], tmp_kv_sb[:, tsl]))
                        _accum(nc, kps[:], mms)
                        nc.vector.tensor_copy(ktT_sb[:, m, sl], kps[:])
                    for m in range(NMH):  # Q^T (prescaled by 1/8 via wqk/lb data)
                        cols = slice(128 * m, 128 * (m + 1))
                        qps = ps_s.tile([128, 512], F32, tag="s")
                        mms = [
                            (wqk_sb[:, ch, cols], xt_s[:, ch, :]) for ch in range(NCH)
                        ]
                        if s >= 2:
                            mms.append((lb_qk_sb[:, cols], tmp_kv_sb[:, tsl]))
                        _accum(nc, qps[:], mms)
                        nc.vector.tensor_copy(qT_sb[:, m, sl], qps[:])
                    for st in range(4):  # V rows (128-row seq tiles)
                        vps = ps_s.tile([128, 512], F32, tag="s")
                        mms = [
                            (
                                xt_s[:, ch, 128 * st : 128 * (st + 1)],
                                wv_sb[:, ch, :],
                            )
                            for ch in range(NCH)
                        ]
                        if s >= 2:
                            base = (s - 2) * 512 + 128 * st
                            mms.append((tmp_kv_sb[:, base : base + 128], lb_v_sb[:]))
                        _accum(nc, vps[:], mms)
                        nc.vector.tensor_copy(
                            v_sb[:, 4 * s + st, :, 0:D],
                            vps[:].rearrange("p (h d) -> p h d", h=8),
                        )

                for s in (0, 2):
                    do_sblock(s)
                nc.sync.dma_start(
                    out=mask_sb[:], in_=masks.rearrange("t p q -> p t q")
                )

                # ===== Phase B/C helpers =====
                wproj_sb = persist.tile([128, NMH, C], BF16)
                for m in range(NMH):
                    nc.sync.dma_start(
                        out=wproj_sb[:, m, :],
                        in_=wproj[128 * m : 128 * (m + 1), :],
                    )

                def _emit_division(half, qb, hp, yus):
                    hoff = half * T + qb * 512
                    for hi in range(2):
                        yu = yus[hi]
                        ysb = pt_pool.tile(
                            [D + 1, 512], F32R, tag="ysb",
                            name=f"ysb_{half}_{qb}_{hp}_{hi}",
                        )
                        nc.vector.tensor_copy(ysb[:], yu[:])
                        r_tmp = small.tile([1, 512], F32R, tag="rtmp")
                        nc.vector.reciprocal(r_tmp[:], ysb[D : D + 1, :])
                        r_bc = ps_s.tile(
                            [128, 512], F32, tag="s",
                            name=f"rbc_{half}_{qb}_{hp}_{hi}",
                        )
                        nc.tensor.matmul(
                            r_bc[:], ones1_sb[:], r_tmp[:], start=True, stop=True
                        )
                        rows = slice(64 * hi, 64 * hi + 64)
                        nc.vector.tensor_mul(
                            y_acc[rows, hp, hoff : hoff + 512],
                            ysb[0:D, :],
                            r_bc[rows, :],
                        )

                def attention_block(half, qb):
                    ktl = _ktiles_for_block(half, qb)
                    nk = len(ktl)
                    pending = []
                    for hp in range(NMH):
                        yus = {}
                        for hi in range(2):
                            yus[hi] = ps_y.tile(
                                [D + 1, 512], F32, tag="y",
                                name=f"yu_{half}_{qb}_{hp}_{hi}",
                            )
                        # 1-deep software pipeline: scores/exp for ktile ki are
                        # emitted before the AV matmuls of ktile ki-1 so the PE
                        # never sits behind an exp it just enabled.
                        prev = None
                        for ki in range(nk + 1):
                            cur = None
                            if ki < nk:
                                region, j = ktl[ki]
                                qts = _active_qts(half, region, j, qb)
                                qlo, qw = qts[0], len(qts)
                                nq = 128 * qw
                                q_sl = slice(
                                    half * T + 128 * qlo,
                                    half * T + 128 * (qlo + qw),
                                )
                                rel_sl = slice(
                                    128 * (qlo - 4 * qb), 128 * (qlo - 4 * qb + qw)
                                )
                                ktidx = (0 if region == "e1" else 8) + j
                                kbase = (0 if region == "e1" else T) + 128 * j
                                mqts = [
                                    qt
                                    for qt in qts
                                    if (half, region, j, qt) in MASK_IDX
                                ]
                                pts = []
                                for hi in range(2):
                                    lo = 64 * hi
                                    sp = ps_s.tile([128, 512], F32, tag="s")
                                    nc.tensor.matmul(
                                        sp[:, 0:nq],
                                        ktT_sb[lo : lo + 64, hp, kbase : kbase + 128],
                                        qT_sb[lo : lo + 64, hp, q_sl],
                                        start=True,
                                        stop=not mqts,
                                    )
                                    for qi, qt in enumerate(mqts):
                                        mi = MASK_IDX[(half, region, j, qt)]
                                        rel = slice(
                                            128 * (qt - qlo), 128 * (qt - qlo + 1)
                                        )
                                        nc.tensor.matmul(
                                            sp[:, rel],
                                            iden_sb[:],
                                            mask_sb[:, mi, :],
                                            start=False,
                                            stop=(qi == len(mqts) - 1),
                                            skip_group_check=True,
                                        )
                                    pt = pt_pool.tile([128, 512], BF16, tag="pt")
                                    nc.scalar.activation(
                                        pt[:, 0:nq],
                                        sp[:, 0:nq],
                                        mybir.ActivationFunctionType.Exp,
                                    )
                                    pts.append(pt)
                                cur = (ki, ktidx, rel_sl, nq, pts)
                                if ki == 1 and pending:
                                    _emit_division(*pending.pop(0))
                            if prev is not None:
                                pki, pktidx, prel, pnq, ppts = prev
                                for hi in range(2):
                                    nc.tensor.matmul(
                                        yus[hi][:, prel],
                                        v_sb[:, pktidx, 2 * hp + hi, :],
                                        ppts[hi][:, 0:pnq],
                                        start=(pki == 0),
                                        stop=(pki == nk - 1),
                                        skip_group_check=True,
                                    )
                            prev = cur
                        pending.append((half, qb, hp, yus))
                    while pending:
                        _emit_division(*pending.pop(0))

                def partial_proj(half, qb):
                    """Partial output projection (my 512 y-cols) for the 512
                    absolute rows (half, qb); stages bf16 into d_in[qb]."""
                    hoff = half * T + qb * 512
                    mms_extra = []
                    if half == 1:  # proj-LoRA mid partial, e2 rows only
                        tm_ps = ps_misc.tile([RANK, 512], F32, tag="s")
                        _accum(
                            nc,
                            tm_ps[:],
                            [
                                (la_proj_sb[:, hp, :], y_acc[:, hp, hoff : hoff + 512])
                                for hp in range(NMH)
                            ],
                        )
                        tm_sb = small.tile([RANK, 512], F32R, tag="tm")
                        nc.vector.tensor_copy(tm_sb[:], tm_ps[:])
                    for st in range(4):
                        row = hoff + 128 * st
                        for co in range(2):
                            cos = slice(512 * co, 512 * (co + 1))
                            ops = ps_s.tile([128, 512], F32, tag="s")
                            mms = [
                                (y_acc[:, hp, row : row + 128], wproj_sb[:, hp, cos])
                                for hp in range(NMH)
                            ]
                            if half == 1:
                                mms.append(
                                    (
                                        tm_sb[:, 128 * st : 128 * (st + 1)],
                                        lb_proj_sb[:, cos],
                                    )
                                )
                            _accum(nc, ops[:], mms)
                            ost = stage.tile([128, 512], BF16, tag="stage")
                            nc.vector.tensor_copy(ost[:], ops[:])
                            nc.sync.dma_start(
                                out=d_in[qb][half, 128 * st : 128 * (st + 1), cos],
                                in_=ost[:],
                            )

                # ---- schedule: interleave attention with remaining
                # s-blocks; RS0 overlaps s3 + qb1 attention ------------------
                attention_block(0, 0)
                partial_proj(0, 0)
                do_sblock(1)
                attention_block(1, 0)
                partial_proj(1, 0)
                nc.gpsimd.collective_compute(
                    "ReduceScatter",
                    mybir.AluOpType.add,
                    replica_groups=REPLICA_GROUPS,
                    ins=[d_in[0][:]],
                    outs=[d_out[0][:]],
                )
                do_sblock(3)
                attention_block(0, 1)
                partial_proj(0, 1)
                attention_block(1, 1)
                partial_proj(1, 1)
                nc.gpsimd.collective_compute(
                    "ReduceScatter",
                    mybir.AluOpType.add,
                    replica_groups=REPLICA_GROUPS,
                    ins=[d_in[1][:]],
                    outs=[d_out[1][:]],
                )

                # ---- receive: direct DRAM->DRAM into the bf16 output ------
                for qb in range(2):
                    nc.sync.dma_start(
                        out=yout[512 * qb : 512 * (qb + 1), :], in_=d_out[qb][:]
                    )
    return nc


_PROGRAM = None


def _get_program():
    global _PROGRAM
    if _PROGRAM is None:
        _PROGRAM = _build_program()
    return _PROGRAM


# ---------------------------------------------------------------------------
# Host side
# ---------------------------------------------------------------------------
def _delayed_mask_np(t):
    ones = np.ones((t, t), dtype=bool)
    m11 = np.tril(ones) & np.triu(ones, -(LOOKAHEAD + OVERLAP))
    m12 = np.tril(ones, -LOOKAHEAD)
    m21 = np.tril(ones, LOOKAHEAD) & np.triu(ones, -OVERLAP)
    m22 = np.tril(ones)
    return np.block([[m11, m12], [m21, m22]])


_MASKS_CACHE = None


def _packed_masks(M):
    global _MASKS_CACHE
    if _MASKS_CACHE is None:
        mk = np.empty((NMASK, 128, 128), dtype=ml_dtypes.bfloat16)
        for i, (half, region, j, qt) in enumerate(MASK_TILES):
            qg = half * T + 128 * qt
            kg = (0 if region == "e1" else T) + 128 * j
            mk[i] = (
                M[qg : qg + 128, kg : kg + 128].T.astype(np.float32) - 1.0
            ) * 40.0
        _MASKS_CACHE = mk
    return _MASKS_CACHE


def _core_inputs(core, e1, e2, W_attn, W_proj, la_attn, lb_attn, la_proj, lb_proj, M):
    b, r = core // 2, core % 2
    f32 = np.float32
    bf16 = ml_dtypes.bfloat16
    hs = slice(512 * r, 512 * r + 512)  # my heads' col block

    x = np.concatenate([e1[b], e2[b]], axis=0)  # [2T, C]
    xT = np.ascontiguousarray(x.T).astype(bf16)

    wqk = np.empty((C, 1024), dtype=f32)
    wqk[:, 0:512] = W_attn[:, hs] * QSCALE
    wqk[:, 512:] = W_attn[:, C + 512 * r : C + 512 * r + 512]
    lb_qk = np.empty((RANK, 1024), dtype=f32)
    lb_qk[:, 0:512] = lb_attn[:, hs] * (LSCALE * QSCALE)
    lb_qk[:, 512:] = lb_attn[:, C + 512 * r : C + 512 * r + 512] * LSCALE

    return {
        "xT": xT,
        "wqk": wqk.astype(bf16),
        "wv": np.ascontiguousarray(
            W_attn[:, 2 * C + 512 * r : 2 * C + 512 * r + 512]
        ).astype(bf16),
        "la_attn": np.ascontiguousarray(la_attn).astype(bf16),
        "lb_qk": lb_qk,
        "lb_v": np.ascontiguousarray(
            lb_attn[:, 2 * C + 512 * r : 2 * C + 512 * r + 512], dtype=f32
        )
        * LSCALE,
        "la_proj": np.ascontiguousarray(la_proj[hs, :]).astype(bf16),
        "lb_proj": np.ascontiguousarray(lb_proj, dtype=f32) * LSCALE,
        "wproj": np.ascontiguousarray(W_proj[hs, :]).astype(bf16),
        "masks": _packed_masks(M),
        "ones1": np.ones((1, 128), dtype=f32),
        "iden": np.eye(128, dtype=f32).astype(bf16),
    }


def kernel(
    e1,
    e2,
    W_attn,
    W_proj,
    lora_A_attn,
    lora_B_attn,
    lora_A_proj,
    lora_B_proj,
    _trace=False,
):
    e1 = np.asarray(e1, np.float32)
    e2 = np.asarray(e2, np.float32)
    nc = _get_program()
    M = _delayed_mask_np(T)
    in_maps = [
        _core_inputs(
            c, e1, e2, W_attn, W_proj, lora_A_attn, lora_B_attn, lora_A_proj,
            lora_B_proj, M,
        )
        for c in range(8)
    ]
    res = run_bass_kernel_spmd(nc, in_maps, core_ids=list(range(8)), trace=_trace)
    y1 = np.stack([res.results[2 * b]["yout"] for b in range(B)])
    y2 = np.stack([res.results[2 * b + 1]["yout"] for b in range(B)])
    if _trace:
        kernel.last_results = res
    return y1, y2
